# revision 1
# baseline (speedup 1.0000x reference)
"""AttnBlock (GroupNorm + self-attn + cross-attn + proj, residual) on 8 trn2 cores.

Sharding: data-parallel over batch B=16 -> 2 images per core; weights replicated.

Per-core layout ("T layout"): feature dim on SBUF partitions, token dim on the
free axis. x arrives as [C, H*W] which already is this layout, so GroupNorm,
all projections, both attentions and the residual run without transposing the
big activations. Only the small weight matrices ([256,256]/[256,512]) and
cemb ([77,512]) are transposed on-chip via the PE.

Matmul operands are bf16 (fp32 PSUM accumulation); softmax logits here are
O(1) by construction (normed activations x 0.02-scale weights, /16), so exp is
computed without max subtraction, and the row-sum denominator is obtained with
an all-ones stationary matmul that also broadcasts it across partitions.
"""

import os

import numpy as np

B, C, H, W, S, CD = 16, 256, 32, 32, 77, 512
HW = H * W
GROUPS = 32
GS = C // GROUPS  # 8 channels per group
EPS = 1e-5
SCALE = C ** (-0.5)  # 1/16
NCORES = 8
BPC = B // NCORES  # batches per core

_CACHE = {}
LAST_RESULT = None  # test harness reads exec_time_ns off this


def _build_nc():
    import concourse.bacc as bacc
    import concourse.bass as bass
    import concourse.tile as tile
    from concourse import mybir

    f32 = mybir.dt.float32
    mm_dt = mybir.dt.bfloat16
    AF = mybir.ActivationFunctionType
    OP = mybir.AluOpType
    AX = mybir.AxisListType

    nc = bacc.Bacc("TRN2", target_bir_lowering=False, debug=False)

    x_d = nc.dram_tensor("x", [BPC, C, HW], f32, kind="ExternalInput")
    # cemb^T and W^T are prepared host-side (transposed + cast to bf16)
    cembT_d = nc.dram_tensor("cembT", [BPC, CD // 128, 128, S], mm_dt,
                             kind="ExternalInput")
    wT_d = {
        name: nc.dram_tensor(
            "wT_" + name, [kin // 128, 128, 2, 128], mm_dt,
            kind="ExternalInput")
        for name, kin in [("wq_s", C), ("wk_s", C), ("wv_s", C), ("wq_c", C),
                          ("w_proj", C), ("wk_c", CD), ("wv_c", CD)]
    }
    vec_d = {
        name: nc.dram_tensor(name, [C], f32, kind="ExternalInput")
        for name in [
            "gn_gamma", "gn_beta", "bq_s", "bk_s", "bv_s",
            "bq_c", "bk_c", "bv_c", "b_proj",
        ]
    }
    y_d = nc.dram_tensor("y", [BPC, C, HW], f32, kind="ExternalOutput")

    def bcast_ap(handle, parts):
        ap = handle[:]
        return bass.AP(tensor=ap.tensor, offset=ap.offset,
                       ap=[[0, parts]] + [list(p) for p in ap.ap])

    with tile.TileContext(nc) as tc:
        with (
            tc.tile_pool(name="const", bufs=1) as const,
            tc.tile_pool(name="work", bufs=2) as work,
            tc.tile_pool(name="heavy", bufs=1) as heavy,
            tc.tile_pool(name="pS", bufs=2, space="PSUM") as pS,
            tc.tile_pool(name="pmm", bufs=4, space="PSUM") as pmm,
        ):
            # ---- constants ----
            ones_mm = const.tile([128, 128], mm_dt)
            nc.vector.memset(ones_mm, 1.0)
            # touch Exp once so its ACT table load overlaps the weight DMAs
            warm = const.tile([128, 1], f32)
            nc.vector.memset(warm, 0.0)
            nc.scalar.activation(warm, warm, AF.Exp)

            # ---- loads on the Sync queue, ordered by when they gate work:
            # x[0] (GroupNorm) -> cross weights + cembT (early PE work) ->
            # qkv weights -> batch-1 activations -> late weights.
            # ---- bias / affine columns: [128, 2] (chunk = high bit of c) ----
            cols = {}
            for name in ["gn_gamma", "gn_beta", "bq_s", "bk_s",
                         "bq_c", "bk_c", "b_proj"]:
                t = const.tile([128, 2], f32, tag=f"col_{name}")
                nc.gpsimd.dma_start(
                    out=t, in_=vec_d[name][:].rearrange("(a p) -> p a", p=128))
                cols[name] = t
            # fold the attention scale into q: bias must be pre-scaled too
            for name in ["bq_s", "bq_c"]:
                nc.vector.tensor_scalar_mul(cols[name], cols[name], SCALE)
            # v biases live on the free axis -> partition-broadcast copies
            bvs_bc = const.tile([128, C], f32)
            nc.gpsimd.dma_start(out=bvs_bc, in_=bcast_ap(vec_d["bv_s"], 128))
            bvc_bc = const.tile([S, C], f32)
            nc.gpsimd.dma_start(out=bvc_bc, in_=bcast_ap(vec_d["bv_c"], S))

            def load_w(name, kin):
                kch = kin // 128
                wt = const.tile([128, kch, 2, 128], mm_dt, tag=f"wT_{name}")
                nc.sync.dma_start(
                    out=wt, in_=wT_d[name][:].rearrange("k p m c -> p k m c"))
                wT[name] = wt

            wT = {}
            xTs, cembTs = [], []
            xT0 = work.tile([128, 2, HW], f32, tag="xT")
            nc.sync.dma_start(
                out=xT0, in_=x_d[0].rearrange("(a p) n -> p a n", p=128))
            xTs.append(xT0)
            cembT0 = work.tile([128, 4, S], mm_dt, tag="cembT")
            nc.sync.dma_start(out=cembT0,
                              in_=cembT_d[0].rearrange("k p s -> p k s"))
            cembTs.append(cembT0)
            load_w("wk_c", CD)
            load_w("wv_c", CD)
            load_w("wq_s", C)
            load_w("wk_s", C)
            load_w("wv_s", C)
            xT1 = work.tile([128, 2, HW], f32, tag="xT")
            nc.sync.dma_start(
                out=xT1, in_=x_d[1].rearrange("(a p) n -> p a n", p=128))
            xTs.append(xT1)
            cembT1 = work.tile([128, 4, S], mm_dt, tag="cembT")
            nc.sync.dma_start(out=cembT1,
                              in_=cembT_d[1].rearrange("k p s -> p k s"))
            cembTs.append(cembT1)
            load_w("wq_c", C)
            load_w("w_proj", C)


            for b in range(BPC):
                xT = xTs[b]
                # ---- cross-attn k/v first: they only need cemb^T, giving
                # the PE work while GroupNorm's stats chain runs ----
                cembT = cembTs[b]
                kcT = work.tile([128, 2, S], mm_dt, tag="kcT")
                for mc in range(2):
                    ps = pmm.tile([128, S], f32, tag="mm")
                    for dc in range(4):
                        nc.tensor.matmul(ps, wT["wk_c"][:, dc, mc, :],
                                         cembT[:, dc, :],
                                         start=(dc == 0), stop=(dc == 3))
                    nc.vector.tensor_scalar_add(kcT[:, mc, :], ps,
                                                cols["bk_c"][:, mc:mc + 1])
                vc_nat = work.tile([S, C], mm_dt, tag="vc_nat")
                ps = pmm.tile([S, C], f32, tag="mm")
                for dc in range(4):
                    nc.tensor.matmul(ps, cembT[:, dc, :], wT["wv_c"][:, dc],
                                     start=(dc == 0), stop=(dc == 3))
                nc.vector.tensor_add(vc_nat, ps, bvc_bc)

                # ---- GroupNorm stats ----
                stats = work.tile([128, 2, 2], f32, tag="stats")
                scratch = heavy.tile([128, HW], f32, tag="scratch")
                for a in range(2):
                    nc.vector.reduce_sum(out=stats[:, a, 0:1], in_=xT[:, a, :],
                                         axis=AX.X)
                    nc.scalar.activation(scratch, xT[:, a, :], AF.Square,
                                         accum_out=stats[:, a, 1:2])
                hnT32 = work.tile([128, 2, HW], f32, tag="hnT32")
                hnmm = work.tile([128, 2, HW], mm_dt, tag="hnmm")
                Acol = work.tile([128, 2], f32, tag="Acol")
                Bcol = work.tile([128, 2], f32, tag="Bcol")
                t1 = work.tile([128, 2], f32, tag="t1")
                # regroup [128(ch), a, s] -> [16(g), 8(ch-in-g), a, s] with
                # one SBUF->SBUF DMA (partition -> free), then reduce over the
                # 8 channels of each group on DVE.
                sg = work.tile([16, 8, 2, 2], f32, tag="sg")
                nc.sync.dma_start(out=sg, in_=stats)
                gsum = work.tile([16, 2, 2], f32, tag="gsum")
                nc.vector.reduce_sum(out=gsum,
                                     in_=sg.rearrange("u w a s -> u a s w"),
                                     axis=AX.X)
                mr = work.tile([16, 2, 2], f32, tag="mr")  # [g, chunk, {mean,rstd}]
                varv = work.tile([16, 2], f32, tag="varv")
                gmv2 = work.tile([16, 2, 2], f32, tag="gmv2")
                nc.vector.tensor_scalar_mul(gmv2, gsum, 1.0 / (GS * HW))
                m2 = work.tile([16, 2], f32, tag="m2")
                nc.vector.tensor_mul(m2, gmv2[:, :, 0], gmv2[:, :, 0])
                nc.vector.tensor_sub(varv, gmv2[:, :, 1], m2)
                nc.vector.tensor_scalar_add(varv, varv, EPS)
                # rstd = rsqrt(var+eps) via Newton on DVE (no ACT table churn);
                # seed 1/v is accurate enough since group var ~= 1 here
                ya = work.tile([16, 2], f32, tag="ya")
                yb = work.tile([16, 2], f32, tag="yb")
                nc.vector.reciprocal_approx_fast(out=ya, in_=varv)
                cur = ya
                for it in range(2):
                    y2 = work.tile([16, 2], f32, tag="y2")
                    nc.vector.tensor_mul(y2, cur, cur)
                    nc.vector.tensor_mul(y2, y2, varv)
                    nc.vector.tensor_scalar(out=y2, in0=y2, scalar1=-0.5,
                                            scalar2=1.5, op0=OP.mult,
                                            op1=OP.add)
                    nxt = yb if cur is ya else ya
                    nc.vector.tensor_mul(nxt, cur, y2)
                    cur = nxt
                nc.vector.tensor_copy(mr[:, :, 0], gmv2[:, :, 0])
                nc.vector.tensor_copy(mr[:, :, 1], cur)
                # broadcast groups back to channels with one DMA (free->partition)
                mrc = work.tile([128, 2, 2], f32, tag="mrc")
                mr_ap = mr[:]
                mr_rep = bass.AP(tensor=mr.tensor, offset=mr_ap.offset,
                                 ap=[list(mr_ap.ap[0]), [0, GS]] +
                                    [list(p) for p in mr_ap.ap[1:]])
                nc.sync.dma_start(out=mrc, in_=mr_rep)
                nc.vector.tensor_mul(Acol, mrc[:, :, 1], cols["gn_gamma"])
                nc.vector.tensor_mul(t1, mrc[:, :, 0], Acol)
                nc.vector.tensor_sub(Bcol, cols["gn_beta"], t1)
                for a in range(2):
                    nc.vector.tensor_scalar(
                        out=hnmm[:, a, :], in0=xT[:, a, :],
                        scalar1=Acol[:, a:a + 1], scalar2=Bcol[:, a:a + 1],
                        op0=OP.mult, op1=OP.add)
                    nc.scalar.activation(
                        out=hnT32[:, a, :], in_=xT[:, a, :], func=AF.Identity,
                        bias=Bcol[:, a:a + 1], scale=Acol[:, a:a + 1])

                # ---- q, k (T layout, scale folded into q) ----
                qT = work.tile([128, 2, HW], mm_dt, tag="qT")
                kT = work.tile([128, 2, HW], mm_dt, tag="kT")
                for wname, bname, dst, sc in [("wq_s", "bq_s", qT, SCALE),
                                              ("wk_s", "bk_s", kT, 1.0)]:
                    for mc in range(2):
                        for nh in range(2):
                            ps = pmm.tile([128, 512], f32, tag="mm")
                            for kc in range(2):
                                nc.tensor.matmul(
                                    ps, wT[wname][:, kc, mc, :],
                                    hnmm[:, kc, nh * 512:(nh + 1) * 512],
                                    start=(kc == 0), stop=(kc == 1))
                            nc.scalar.activation(
                                out=dst[:, mc, nh * 512:(nh + 1) * 512],
                                in_=ps, func=AF.Identity,
                                bias=cols[bname][:, mc:mc + 1], scale=sc)

                # ---- v in natural layout [m(part chunks), c'] ----
                v_nat = work.tile([128, 8, C], mm_dt, tag="v_nat")
                for m8 in range(8):
                    ps = pS.tile([128, C], f32, tag="pS")
                    for kc in range(2):
                        nc.tensor.matmul(
                            ps, hnmm[:, kc, m8 * 128:(m8 + 1) * 128],
                            wT["wv_s"][:, kc], start=(kc == 0), stop=(kc == 1))
                    nc.vector.tensor_add(v_nat[:, m8, :], ps, bvs_bc)

                # ---- S^T = k q^T (already scaled), exp ----
                expST = heavy.tile([128, 8, HW], mm_dt, tag="expST")
                for m8 in range(8):
                    ps = pS.tile([128, HW], f32, tag="pS")
                    for nh in range(2):
                        for kc in range(2):
                            nc.tensor.matmul(
                                ps[:, nh * 512:(nh + 1) * 512],
                                kT[:, kc, m8 * 128:(m8 + 1) * 128],
                                qT[:, kc, nh * 512:(nh + 1) * 512],
                                start=(kc == 0), stop=(kc == 1))
                    nc.scalar.activation(expST[:, m8, :], ps, AF.Exp)

                # ---- row sums: one level of pairwise adds on DVE, then an
                # ones-stationary matmul reduces 4 chunks + broadcasts ----
                psum4 = work.tile([128, 4, HW], mm_dt, tag="psum4")
                for i in range(4):
                    nc.vector.tensor_add(psum4[:, i, :], expST[:, 2 * i, :],
                                         expST[:, 2 * i + 1, :])
                rinv = work.tile([128, HW], f32, tag="rinv")
                for nh in range(2):
                    ps = pmm.tile([128, 512], f32, tag="mm")
                    for i in range(4):
                        nc.tensor.matmul(
                            ps, ones_mm, psum4[:, i, nh * 512:(nh + 1) * 512],
                            start=(i == 0), stop=(i == 3))
                    nc.vector.reciprocal_approx_fast(
                        out=rinv[:, nh * 512:(nh + 1) * 512], in_=ps)

                # ---- U = expS^T-weighted V, h2 = hn + U * rinv ----
                h2T = work.tile([128, 2, HW], mm_dt, tag="h2T")
                tmp = work.tile([128, 512], f32, tag="tmp")
                for mc in range(2):
                    for nh in range(2):
                        ps = pmm.tile([128, 512], f32, tag="mm")
                        for m8 in range(8):
                            nc.tensor.matmul(
                                ps, v_nat[:, m8, mc * 128:(mc + 1) * 128],
                                expST[:, m8, nh * 512:(nh + 1) * 512],
                                start=(m8 == 0), stop=(m8 == 7))
                        nc.vector.tensor_tensor(
                            tmp, ps, rinv[:, nh * 512:(nh + 1) * 512],
                            op=OP.mult)
                        nc.vector.tensor_add(
                            h2T[:, mc, nh * 512:(nh + 1) * 512], tmp,
                            hnT32[:, mc, nh * 512:(nh + 1) * 512])

                # ---- qc (scaled), S_c^T, exp, rowsums, hc ----
                qcT = work.tile([128, 2, HW], mm_dt, tag="qcT")
                for mc in range(2):
                    for nh in range(2):
                        ps = pmm.tile([128, 512], f32, tag="mm")
                        for kc in range(2):
                            nc.tensor.matmul(
                                ps, wT["wq_c"][:, kc, mc, :],
                                h2T[:, kc, nh * 512:(nh + 1) * 512],
                                start=(kc == 0), stop=(kc == 1))
                        nc.scalar.activation(
                            out=qcT[:, mc, nh * 512:(nh + 1) * 512],
                            in_=ps, func=AF.Identity,
                            bias=cols["bq_c"][:, mc:mc + 1], scale=SCALE)
                expScT = work.tile([S, HW], mm_dt, tag="expScT")
                psc = pS.tile([S, HW], f32, tag="pS")
                for nh in range(2):
                    for kc in range(2):
                        nc.tensor.matmul(
                            psc[:, nh * 512:(nh + 1) * 512], kcT[:, kc, :],
                            qcT[:, kc, nh * 512:(nh + 1) * 512],
                            start=(kc == 0), stop=(kc == 1))
                nc.scalar.activation(expScT, psc, AF.Exp)
                rcinv = work.tile([128, HW], f32, tag="rcinv")
                for nh in range(2):
                    ps = pmm.tile([128, 512], f32, tag="mm")
                    nc.tensor.matmul(ps, ones_mm[:S, :],
                                     expScT[:, nh * 512:(nh + 1) * 512],
                                     start=True, stop=True)
                    nc.vector.reciprocal_approx_fast(
                        out=rcinv[:, nh * 512:(nh + 1) * 512], in_=ps)
                hcT = work.tile([128, 2, HW], mm_dt, tag="hcT")
                for mc in range(2):
                    for nh in range(2):
                        ps = pmm.tile([128, 512], f32, tag="mm")
                        nc.tensor.matmul(
                            ps, vc_nat[:, mc * 128:(mc + 1) * 128],
                            expScT[:, nh * 512:(nh + 1) * 512],
                            start=True, stop=True)
                        nc.vector.tensor_tensor(
                            hcT[:, mc, nh * 512:(nh + 1) * 512], ps,
                            rcinv[:, nh * 512:(nh + 1) * 512], op=OP.mult)

                # ---- final projection + bias + residual ----
                y_sb = work.tile([128, 2, HW], f32, tag="y_sb")
                for mc in range(2):
                    for nh in range(2):
                        ps = pmm.tile([128, 512], f32, tag="mm")
                        for kc in range(2):
                            nc.tensor.matmul(
                                ps, wT["w_proj"][:, kc, mc, :],
                                hcT[:, kc, nh * 512:(nh + 1) * 512],
                                start=(kc == 0), stop=(kc == 1))
                        nc.vector.scalar_tensor_tensor(
                            out=y_sb[:, mc, nh * 512:(nh + 1) * 512],
                            in0=ps, scalar=cols["b_proj"][:, mc:mc + 1],
                            in1=xT[:, mc, nh * 512:(nh + 1) * 512],
                            op0=OP.add, op1=OP.add)
                for mc in range(2):
                    nc.sync.dma_start(
                        out=y_d[b].rearrange("(a p) n -> p a n", p=128)[:, mc, :],
                        in_=y_sb[:, mc, :])

    nc.finalize()
    return nc


def host_inputs(inputs):
    import ml_dtypes
    bf16 = ml_dtypes.bfloat16
    f = lambda a: np.ascontiguousarray(np.asarray(a, dtype=np.float32))
    x = f(inputs["x"]).reshape(B, C, HW)
    # cemb^T in bf16: [B, CD/128, 128, S]
    cembT = np.ascontiguousarray(
        f(inputs["cemb"]).transpose(0, 2, 1).reshape(B, CD // 128, 128, S)
    ).astype(bf16)
    shared = {
        name: f(inputs[name])
        for name in ["gn_gamma", "gn_beta", "bq_s", "bk_s", "bv_s",
                     "bq_c", "bk_c", "bv_c", "b_proj"]
    }
    # W^T in bf16, tiled [kin/128, 128, 2, 128]
    for name in ["wq_s", "wk_s", "wv_s", "wq_c", "w_proj", "wk_c", "wv_c"]:
        w = f(inputs[name])
        kin = w.shape[1]
        shared["wT_" + name] = np.ascontiguousarray(
            w.T.reshape(kin // 128, 128, 2, 128)).astype(bf16)
    return [
        {"x": x[i * BPC:(i + 1) * BPC], "cembT": cembT[i * BPC:(i + 1) * BPC],
         **shared}
        for i in range(NCORES)
    ]


def kernel(**inputs):
    global LAST_RESULT
    from concourse.bass_utils import run_bass_kernel_spmd

    if "nc" not in _CACHE:
        _CACHE["nc"] = _build_nc()
    nc = _CACHE["nc"]

    in_maps = host_inputs(inputs)
    res = run_bass_kernel_spmd(nc, in_maps, list(range(NCORES)),
                               trace=bool(os.environ.get("BASS_TRACE")))
    LAST_RESULT = res
    y = np.concatenate([res.results[i]["y"] for i in range(NCORES)], axis=0)
    return y.reshape(B, C, H, W).astype(np.float32)



# revision 5
# speedup vs baseline: 1.0465x; 1.0465x over previous
"""AttnBlock (GroupNorm + self-attn + cross-attn + proj, residual) on 8 trn2 cores.

Sharding: data-parallel over batch B=16 -> 2 images per core; weights replicated.

v2: fp8e4 DoubleRow matmuls (K=256 per MM) for every K>=256 contraction, which
halves the PE instruction count and nearly doubles matmul throughput. All
weights are pre-scaled x16 host-side so their fp8 encoding stays out of the
subnormal range; descales ride the (otherwise free) affine slots of the PSUM
evacuation ops. Softmax normalization for cross-attn is applied to the (tiny)
[77, HW] weight matrix before the value matmul so the hc/proj chain needs no
per-token divide; the self-attn divide is fused with the residual via one
tensor_tensor + one scalar_tensor_tensor. GroupNorm stats use a single
bn_stats pass (DVE) instead of reduce_sum + Square. The two images per core
are stage-interleaved so each engine works on image b+1 while the next engine
consumes image b.

Scale ledger (host WS=16 on all weights):
  qT = 2(q+bq)   kT = 2(k+bk)     -> logits' = 4*logits, exp(scale=1/64)
  v' = 2*v0 (no bias), ones_self=2 -> rinv = 1/(2r), tmp = U/r exact
  h2 = hn + tmp + bv_s (stt)
  qcT = 2(qc+bqc), kcT = 2(kc+bkc) -> exp(scale=1/64)
  ones_cross = 1/8 -> rcinv = 8/r, expScN = 8*w_norm, vc_nat = vc0+bvc
  hcT = 8*hc; proj psum = 128*(Wp hc + bp) (bias via K=1 ones matmul)
  y = psum/128 + x (stt)
"""

import os

import numpy as np

B, C, H, W, S, CD = 16, 256, 32, 32, 77, 512
HW = H * W
SP = 80  # S padded to a 16B-aligned stride for DoubleRow APs
GROUPS = 32
GS = C // GROUPS
EPS = 1e-5
NCORES = 8
BPC = B // NCORES

WS = 16.0          # host-side weight scale (fp8 subnormal avoidance)
QS = 2.0           # q/k/qc/kc storage scale
EXPS = 1.0 / (16.0 * QS * QS)  # exp scale: logits' = QS^2 * q.k, want q.k/16
VSC = 2.0          # v storage scale == ones_self value
HCS = 8.0          # hc storage scale; ones_cross = 1/HCS
PD = 1.0 / (WS * HCS)  # proj psum descale

_CACHE = {}
LAST_RESULT = None  # test harness reads exec_time_ns off this


def _build_nc():
    import concourse.bacc as bacc
    import concourse.bass as bass
    import concourse.tile as tile
    from concourse import mybir

    f32 = mybir.dt.float32
    bf16 = mybir.dt.bfloat16
    fp8 = mybir.dt.float8e4
    AF = mybir.ActivationFunctionType
    OP = mybir.AluOpType
    DR = mybir.MatmulPerfMode.DoubleRow

    nc = bacc.Bacc("TRN2", target_bir_lowering=False, debug=False)

    x_d = nc.dram_tensor("x", [BPC, C, HW], f32, kind="ExternalInput")
    cembT_d = nc.dram_tensor("cembT", [BPC, CD // 128, 128, SP], fp8,
                             kind="ExternalInput")
    wT_d = {
        name: nc.dram_tensor(
            "wT_" + name, [kin // 128, 128, 2, 128], fp8,
            kind="ExternalInput")
        for name, kin in [("wq_s", C), ("wk_s", C), ("wv_s", C), ("wq_c", C),
                          ("w_proj", C), ("wk_c", CD), ("wv_c", CD)]
    }
    vec_d = {
        name: nc.dram_tensor(name, [C], f32, kind="ExternalInput")
        for name in [
            "gn_gamma", "gn_beta", "bq_s2", "bk_s2", "bv_s",
            "bq_c2", "bk_c2", "bv_c",
        ]
    }
    bp_d = nc.dram_tensor("bp_row", [1, C], bf16, kind="ExternalInput")
    y_d = nc.dram_tensor("y", [BPC, C, HW], f32, kind="ExternalOutput")

    def bcast_ap(handle, parts):
        ap = handle[:]
        return bass.AP(tensor=ap.tensor, offset=ap.offset,
                       ap=[[0, parts]] + [list(p) for p in ap.ap])

    with tile.TileContext(nc) as tc:
        with (
            tc.tile_pool(name="const", bufs=1) as const,
            tc.tile_pool(name="work", bufs=2) as work,
            tc.tile_pool(name="psp", bufs=4, space="PSUM") as psp,
        ):
            # ---- constants ----
            ones2 = const.tile([128, 2, 128], fp8)
            nc.vector.memset(ones2, VSC)
            onesc = const.tile([S, 128], bf16)
            nc.vector.memset(onesc, 1.0 / HCS)
            ones_row = const.tile([1, 512], bf16)
            nc.vector.memset(ones_row, 1.0)
            bp_sb = const.tile([1, C], bf16)
            nc.sync.dma_start(out=bp_sb, in_=bp_d[:])
            # touch Exp once so its ACT table load overlaps the weight DMAs
            warm = const.tile([128, 1], f32)
            nc.vector.memset(warm, 0.0)
            nc.scalar.activation(warm, warm, AF.Exp)

            cols = {}
            for name in ["gn_gamma", "gn_beta", "bq_s2", "bk_s2", "bv_s",
                         "bq_c2", "bk_c2"]:
                t = const.tile([128, 2], f32, tag=f"col_{name}")
                nc.gpsimd.dma_start(
                    out=t, in_=vec_d[name][:].rearrange("(a p) -> p a", p=128))
                cols[name] = t
            bvc_bc = const.tile([S, C], f32)
            nc.gpsimd.dma_start(out=bvc_bc, in_=bcast_ap(vec_d["bv_c"], S))

            def load_w(name, kin):
                kch = kin // 128
                wt = const.tile([128, kch, 2, 128], fp8, tag=f"wT_{name}")
                nc.sync.dma_start(
                    out=wt, in_=wT_d[name][:].rearrange("k p m c -> p k m c"))
                wT[name] = wt

            wT = {}
            xTs, cembTs = [], []
            xT0 = work.tile([128, 2, HW], f32, tag="xT")
            nc.sync.dma_start(
                out=xT0, in_=x_d[0].rearrange("(a p) n -> p a n", p=128))
            xTs.append(xT0)
            cembT0 = work.tile([128, 4, SP], fp8, tag="cembT")
            nc.sync.dma_start(out=cembT0,
                              in_=cembT_d[0].rearrange("k p s -> p k s"))
            cembTs.append(cembT0)
            load_w("wk_c", CD)
            load_w("wv_c", CD)
            load_w("wq_s", C)
            load_w("wk_s", C)
            load_w("wv_s", C)
            xT1 = work.tile([128, 2, HW], f32, tag="xT")
            nc.sync.dma_start(
                out=xT1, in_=x_d[1].rearrange("(a p) n -> p a n", p=128))
            xTs.append(xT1)
            cembT1 = work.tile([128, 4, SP], fp8, tag="cembT")
            nc.sync.dma_start(out=cembT1,
                              in_=cembT_d[1].rearrange("k p s -> p k s"))
            cembTs.append(cembT1)
            load_w("wq_c", C)
            load_w("w_proj", C)

            wvs_flat = wT["wv_s"][:].rearrange("p k m c -> p k (m c)")
            wvc_flat = wT["wv_c"][:].rearrange("p k m c -> p k (m c)")

            nb = lambda ap, nh: ap[:, nh * 512:(nh + 1) * 512]

            # per-image tile dicts
            T = [dict(xT=xTs[b], cembT=cembTs[b]) for b in range(BPC)]
            for b in range(BPC):
                t = T[b]
                t["kcT"] = work.tile([128, 2, SP], fp8, tag="kcT", name="kcT")
                t["vc_nat"] = work.tile([S, C], bf16, tag="vc_nat", name="vc_nat")
                t["stats6"] = work.tile([128, 2, 2, 6], f32, tag="stats6", name="stats6")
                t["sg"] = work.tile([16, 8, 2, 2, 6], f32, tag="sg", name="sg")
                t["gout"] = work.tile([16, 2, 2], f32, tag="gout", name="gout")
                t["varv"] = work.tile([16, 2], f32, tag="varv", name="varv")
                t["ya"] = work.tile([16, 2], f32, tag="ya", name="ya")
                t["yb"] = work.tile([16, 2], f32, tag="yb", name="yb")
                t["y2"] = work.tile([16, 2], f32, tag="y2", name="y2")
                t["mr"] = work.tile([16, 2, 2], f32, tag="mr", name="mr")
                t["mrc"] = work.tile([128, 2, 2], f32, tag="mrc", name="mrc")
                t["Acol"] = work.tile([128, 2], f32, tag="Acol", name="Acol")
                t["Bcol"] = work.tile([128, 2], f32, tag="Bcol", name="Bcol")
                t["t1"] = work.tile([128, 2], f32, tag="t1", name="t1")
                t["hnmm"] = work.tile([128, 2, HW], fp8, tag="hnmm", name="hnmm")
                t["qT"] = work.tile([128, 2, HW], fp8, tag="qT", name="qT")
                t["kT"] = work.tile([128, 2, HW], fp8, tag="kT", name="kT")
                t["v_nat"] = work.tile([128, 8, C], fp8, tag="v_nat", name="v_nat")
                t["expST"] = work.tile([128, 8, HW], fp8, tag="expST", name="expST")
                t["rinv"] = work.tile([128, HW], f32, tag="rinv", name="rinv")
                t["tmp"] = work.tile([128, 2, HW], bf16, tag="tmp", name="tmp")
                t["h2T"] = work.tile([128, 2, HW], fp8, tag="h2T", name="h2T")
                t["qcT"] = work.tile([128, 2, HW], fp8, tag="qcT", name="qcT")
                t["expScT"] = work.tile([S, HW], bf16, tag="expScT", name="expScT")
                t["expScN"] = work.tile([S, HW], bf16, tag="expScN", name="expScN")
                t["rcinv"] = work.tile([128, HW], f32, tag="rcinv", name="rcinv")
                t["hcT"] = work.tile([128, 2, HW], fp8, tag="hcT", name="hcT")
                t["y_sb"] = work.tile([128, 2, HW], f32, tag="y_sb", name="y_sb")

            ps = lambda: psp.tile([128, HW], f32, tag="ps", name="ps")

            # ---- stage 1: GroupNorm stats (DVE) ----
            for b in range(BPC):
                t = T[b]
                for a in range(2):
                    for ch in range(2):
                        nc.vector.bn_stats(
                            t["stats6"][:, a, ch, :],
                            t["xT"][:, a, ch * 512:(ch + 1) * 512])
                nc.sync.dma_start(out=t["sg"], in_=t["stats6"])
            for b in range(BPC):
                t = T[b]
                for a in range(2):
                    nc.vector.bn_aggr(t["gout"][:, a, :], t["sg"][:, :, a, :, :])
                nc.vector.tensor_scalar_add(t["varv"], t["gout"][:, :, 1], EPS)
                # rstd = rsqrt(var+eps): 1/v seed + 2 Newton iterations
                nc.vector.reciprocal_approx_fast(out=t["ya"], in_=t["varv"])
                cur = t["ya"]
                for it in range(2):
                    nc.vector.tensor_mul(t["y2"], cur, cur)
                    nc.vector.tensor_mul(t["y2"], t["y2"], t["varv"])
                    nc.vector.tensor_scalar(out=t["y2"], in0=t["y2"],
                                            scalar1=-0.5, scalar2=1.5,
                                            op0=OP.mult, op1=OP.add)
                    nxt = t["yb"] if cur is t["ya"] else t["ya"]
                    nc.vector.tensor_mul(nxt, cur, t["y2"])
                    cur = nxt
                nc.vector.tensor_copy(t["mr"][:, :, 0], t["gout"][:, :, 0])
                nc.vector.tensor_copy(t["mr"][:, :, 1], cur)
                mr_ap = t["mr"][:]
                mr_rep = bass.AP(tensor=t["mr"].tensor, offset=mr_ap.offset,
                                 ap=[list(mr_ap.ap[0]), [0, GS]] +
                                    [list(p) for p in mr_ap.ap[1:]])
                nc.sync.dma_start(out=t["mrc"], in_=mr_rep)

            # ---- stage 2: cross k/v matmuls (PE warms during GN chain) ----
            for b in range(BPC):
                t = T[b]
                kc_ps = ps()
                for mc in range(2):
                    for i in range(2):
                        nc.tensor.matmul(
                            kc_ps[:, mc * 512:mc * 512 + SP],
                            wT["wk_c"][:, 2 * i:2 * i + 2, mc, :],
                            t["cembT"][:, 2 * i:2 * i + 2, :],
                            start=(i == 0), stop=(i == 1), perf_mode=DR)
                nc.vector.memset(t["kcT"][:, :, S:SP], 0.0)
                for mc in range(2):
                    nc.scalar.activation(
                        out=t["kcT"][:, mc, 0:S],
                        in_=kc_ps[:, mc * 512:mc * 512 + S], func=AF.Identity,
                        bias=cols["bk_c2"][:, mc:mc + 1], scale=QS / WS)
                vc_ps = ps()
                for i in range(2):
                    nc.tensor.matmul(
                        vc_ps[0:SP, 0:C],
                        t["cembT"][:, 2 * i:2 * i + 2, :],
                        wvc_flat[:, 2 * i:2 * i + 2, :],
                        start=(i == 0), stop=(i == 1), perf_mode=DR)
                nc.vector.scalar_tensor_tensor(
                    out=t["vc_nat"], in0=vc_ps[0:S, 0:C], scalar=1.0 / WS,
                    in1=bvc_bc, op0=OP.mult, op1=OP.add)

            # ---- stage 3: GN affine + hnmm ----
            for b in range(BPC):
                t = T[b]
                nc.vector.tensor_mul(t["Acol"], t["mrc"][:, :, 1],
                                     cols["gn_gamma"])
                nc.vector.tensor_mul(t["t1"], t["mrc"][:, :, 0], t["Acol"])
                nc.vector.tensor_sub(t["Bcol"], cols["gn_beta"], t["t1"])
                for a in range(2):
                    nc.vector.tensor_scalar(
                        out=t["hnmm"][:, a, :], in0=t["xT"][:, a, :],
                        scalar1=t["Acol"][:, a:a + 1],
                        scalar2=t["Bcol"][:, a:a + 1],
                        op0=OP.mult, op1=OP.add)

            # ---- stage 4: q, k projections ----
            for b in range(BPC):
                t = T[b]
                for wname, bname, dst in [("wq_s", "bq_s2", t["qT"]),
                                          ("wk_s", "bk_s2", t["kT"])]:
                    for mc in range(2):
                        qp = ps()
                        for nh in range(2):
                            nc.tensor.matmul(
                                nb(qp, nh), wT[wname][:, :, mc, :],
                                t["hnmm"][:, :, nh * 512:(nh + 1) * 512],
                                start=True, stop=True, perf_mode=DR)
                        nc.scalar.activation(
                            out=dst[:, mc, :], in_=qp, func=AF.Identity,
                            bias=cols[bname][:, mc:mc + 1], scale=QS / WS)

            # ---- stage 5: v projection ----
            for b in range(BPC):
                t = T[b]
                for half in range(2):
                    vp = ps()
                    for j in range(4):
                        m8 = 4 * half + j
                        nc.tensor.matmul(
                            vp[:, j * 256:(j + 1) * 256],
                            t["hnmm"][:, :, m8 * 128:(m8 + 1) * 128],
                            wvs_flat,
                            start=True, stop=True, perf_mode=DR)
                    nc.scalar.mul(
                        t["v_nat"][:, 4 * half:4 * half + 4, :],
                        vp[:].rearrange("p (j c) -> p j c", c=256), VSC / WS)

            # ---- stage 6: S^T + exp ----
            for b in range(BPC):
                t = T[b]
                for m8 in range(8):
                    sp = ps()
                    for nh in range(2):
                        nc.tensor.matmul(
                            nb(sp, nh), t["kT"][:, :, m8 * 128:(m8 + 1) * 128],
                            t["qT"][:, :, nh * 512:(nh + 1) * 512],
                            start=True, stop=True, perf_mode=DR)
                    nc.scalar.activation(t["expST"][:, m8, :], sp, AF.Exp,
                                         scale=EXPS)

            # ---- stage 7: rowsum + rinv ----
            for b in range(BPC):
                t = T[b]
                rp = ps()
                for nh in range(2):
                    for i in range(4):
                        nc.tensor.matmul(
                            nb(rp, nh), ones2,
                            t["expST"][:, 2 * i:2 * i + 2,
                                       nh * 512:(nh + 1) * 512],
                            start=(i == 0), stop=(i == 3), perf_mode=DR)
                nc.vector.reciprocal_approx_fast(out=t["rinv"], in_=rp)

            # ---- stage 8: attn@V + h2 ----
            for b in range(BPC):
                t = T[b]
                aps = [ps(), ps()]
                for i in range(4):
                    for mc in range(2):
                        for nh in range(2):
                            nc.tensor.matmul(
                                nb(aps[mc], nh),
                                t["v_nat"][:, 2 * i:2 * i + 2,
                                           mc * 128:(mc + 1) * 128],
                                t["expST"][:, 2 * i:2 * i + 2,
                                           nh * 512:(nh + 1) * 512],
                                start=(i == 0), stop=(i == 3), perf_mode=DR)
                for mc in range(2):
                    nc.vector.tensor_tensor(t["tmp"][:, mc, :], aps[mc],
                                            t["rinv"], op=OP.mult)
                    nc.vector.scalar_tensor_tensor(
                        out=t["h2T"][:, mc, :], in0=t["tmp"][:, mc, :],
                        scalar=cols["bv_s"][:, mc:mc + 1],
                        in1=t["hnmm"][:, mc, :], op0=OP.add, op1=OP.add)

            # ---- stage 9: qc ----
            for b in range(BPC):
                t = T[b]
                for mc in range(2):
                    qp = ps()
                    for nh in range(2):
                        nc.tensor.matmul(
                            nb(qp, nh), wT["wq_c"][:, :, mc, :],
                            t["h2T"][:, :, nh * 512:(nh + 1) * 512],
                            start=True, stop=True, perf_mode=DR)
                    nc.scalar.activation(
                        out=t["qcT"][:, mc, :], in_=qp, func=AF.Identity,
                        bias=cols["bq_c2"][:, mc:mc + 1], scale=QS / WS)

            # ---- stage 10: Sc + exp + rcinv + normalized weights ----
            for b in range(BPC):
                t = T[b]
                scp = ps()
                for nh in range(2):
                    nc.tensor.matmul(
                        scp[0:SP, nh * 512:(nh + 1) * 512], t["kcT"][:],
                        t["qcT"][:, :, nh * 512:(nh + 1) * 512],
                        start=True, stop=True, perf_mode=DR)
                nc.scalar.activation(t["expScT"], scp[0:S, :], AF.Exp,
                                     scale=EXPS)
            for b in range(BPC):
                t = T[b]
                crp = ps()
                for nh in range(2):
                    nc.tensor.matmul(
                        nb(crp, nh), onesc,
                        t["expScT"][:, nh * 512:(nh + 1) * 512],
                        start=True, stop=True)
                nc.vector.reciprocal_approx_fast(out=t["rcinv"], in_=crp)
                nc.vector.tensor_tensor(t["expScN"], t["expScT"],
                                        t["rcinv"][0:S, :], op=OP.mult)

            # ---- stage 11: hc ----
            for b in range(BPC):
                t = T[b]
                for mc in range(2):
                    hp = ps()
                    for nh in range(2):
                        nc.tensor.matmul(
                            nb(hp, nh),
                            t["vc_nat"][:, mc * 128:(mc + 1) * 128],
                            t["expScN"][:, nh * 512:(nh + 1) * 512],
                            start=True, stop=True)
                    nc.scalar.copy(t["hcT"][:, mc, :], hp)

            # ---- stage 12: proj + bias + residual, store ----
            for b in range(BPC):
                t = T[b]
                for mc in range(2):
                    pp = ps()
                    for nh in range(2):
                        nc.tensor.matmul(
                            nb(pp, nh), bp_sb[0:1, mc * 128:(mc + 1) * 128],
                            ones_row[0:1, :],
                            start=True, stop=False, skip_group_check=True)
                        nc.tensor.matmul(
                            nb(pp, nh), wT["w_proj"][:, :, mc, :],
                            t["hcT"][:, :, nh * 512:(nh + 1) * 512],
                            start=False, stop=True, perf_mode=DR,
                            skip_group_check=True)
                    nc.vector.scalar_tensor_tensor(
                        out=t["y_sb"][:, mc, :], in0=pp, scalar=PD,
                        in1=t["xT"][:, mc, :], op0=OP.mult, op1=OP.add)
                    nc.sync.dma_start(
                        out=y_d[b].rearrange("(a p) n -> p a n",
                                             p=128)[:, mc, :],
                        in_=t["y_sb"][:, mc, :])

    nc.finalize()
    return nc


def host_inputs(inputs):
    import ml_dtypes
    bf16 = ml_dtypes.bfloat16
    fp8 = ml_dtypes.float8_e4m3
    f = lambda a: np.ascontiguousarray(np.asarray(a, dtype=np.float32))
    x = f(inputs["x"]).reshape(B, C, HW)
    cembT = np.zeros((B, CD // 128, 128, SP), np.float32)
    cembT[:, :, :, :S] = f(inputs["cemb"]).transpose(0, 2, 1).reshape(
        B, CD // 128, 128, S)
    cembT = cembT.astype(fp8)
    shared = {
        "gn_gamma": f(inputs["gn_gamma"]),
        "gn_beta": f(inputs["gn_beta"]),
        "bv_s": f(inputs["bv_s"]),
        "bv_c": f(inputs["bv_c"]),
        "bq_s2": QS * f(inputs["bq_s"]),
        "bk_s2": QS * f(inputs["bk_s"]),
        "bq_c2": QS * f(inputs["bq_c"]),
        "bk_c2": QS * f(inputs["bk_c"]),
        "bp_row": np.ascontiguousarray(
            (WS * HCS * f(inputs["b_proj"])).reshape(1, C)).astype(bf16),
    }
    for name in ["wq_s", "wk_s", "wv_s", "wq_c", "w_proj", "wk_c", "wv_c"]:
        w = f(inputs[name])
        kin = w.shape[1]
        shared["wT_" + name] = np.ascontiguousarray(
            (WS * w.T).reshape(kin // 128, 128, 2, 128)).astype(fp8)
    return [
        {"x": x[i * BPC:(i + 1) * BPC], "cembT": cembT[i * BPC:(i + 1) * BPC],
         **shared}
        for i in range(NCORES)
    ]


def kernel(**inputs):
    global LAST_RESULT
    from concourse.bass_utils import run_bass_kernel_spmd

    if "nc" not in _CACHE:
        _CACHE["nc"] = _build_nc()
    nc = _CACHE["nc"]

    in_maps = host_inputs(inputs)
    res = run_bass_kernel_spmd(nc, in_maps, list(range(NCORES)),
                               trace=bool(os.environ.get("BASS_TRACE")))
    LAST_RESULT = res
    y = np.concatenate([res.results[i]["y"] for i in range(NCORES)], axis=0)
    return y.reshape(B, C, H, W).astype(np.float32)
